# revision 7
# baseline (speedup 1.0000x reference)
"""Multi-head attention (B=4, S=2048, D=1024, H=16, HS=64, causal) on 8 trn2 cores.

Strategy: tensor-parallel over heads (2 heads per core), x replicated.
Per core: QKV projections (fp32r matmuls), causal attention with
transposed-scores softmax (no max-subtraction; scores are O(1) by
construction), output projection of the local head pair -> partial
[B*S, D]. Host sums the 8 partials (the Wo contraction over heads).

All matmuls run in float32r (fp32 with 12-bit-significand rounding,
bit-compatible with fp32) which streams at 1 cycle/row like bf16 when
the moving free dim >= 256. Host pre-rounds all matmul inputs.
"""

import sys

sys.path.insert(0, "/opt/trn_rl_repo")

import numpy as np

import concourse.bacc as bacc
import concourse.bass as bass
import concourse.mybir as mybir
import concourse.tile as tile
from concourse.bass_utils import run_bass_kernel_spmd

F32 = mybir.dt.float32
F32R = mybir.dt.float32r
EXP = mybir.ActivationFunctionType.Exp
MUL = mybir.AluOpType.mult

B, S, D, H, HS = 4, 2048, 1024, 16, 64
ROWS = B * S                      # 8192
NB = 8                            # cores
SCALE = 1.0 / float(np.sqrt(HS))  # 0.125
CH = 512                          # row/q chunk and matmul moving width
NC_CH = S // CH                   # 4 chunks per batch
NKT = S // 128                    # 16 k-tiles per batch
NQT = S // 128                    # 16 q(row)-tiles per batch

# knobs for test.py
TRACE = False
LAST_RESULTS = None
LAST_IN_MAPS = None


def round_fp32r(a: np.ndarray) -> np.ndarray:
    """Round fp32 to fp32r (12-bit significand, round-to-nearest-even)."""
    u = np.ascontiguousarray(a, dtype=np.float32).view(np.uint32)
    low = u & np.uint32(0xFFF)
    hi = u >> np.uint32(12)
    rnd = (low > 0x800) | ((low == 0x800) & ((hi & 1) == 1))
    return ((hi + rnd.astype(np.uint32)) << np.uint32(12)).view(np.float32)


def _build_nc(reps: int = 1):
    nc = bacc.Bacc()

    xT = nc.declare_dram_parameter("xT", [D, ROWS], F32R, isOutput=False)
    wq = nc.declare_dram_parameter("wq", [D, 128], F32R, isOutput=False)
    wk = nc.declare_dram_parameter("wk", [D, 128], F32R, isOutput=False)
    wv = nc.declare_dram_parameter("wv", [D, 128], F32R, isOutput=False)
    wo0 = nc.declare_dram_parameter("wo0", [64, D], F32R, isOutput=False)
    wo1 = nc.declare_dram_parameter("wo1", [64, D], F32R, isOutput=False)
    bq = nc.declare_dram_parameter("bq", [128, 1], F32, isOutput=False)
    bk = nc.declare_dram_parameter("bk", [128, 1], F32, isOutput=False)
    bv = nc.declare_dram_parameter("bv", [128, 1], F32, isOutput=False)
    masks = nc.declare_dram_parameter("masks", [128, 4 * CH], F32R, isOutput=False)
    ident = nc.declare_dram_parameter("ident", [128, 128], F32R, isOutput=False)
    ones = nc.declare_dram_parameter("ones", [128, 128], F32R, isOutput=False)
    out = nc.declare_dram_parameter("out", [ROWS, D], F32, isOutput=True)

    with tile.TileContext(nc) as tc:
        with tc.tile_pool(name="const", bufs=1) as cpool, \
             tc.tile_pool(name="sb", bufs=1) as sb, \
             tc.tile_pool(name="ps", bufs=1, space="PSUM") as ps:

            # ---- persistent constants ----
            wq_sb = cpool.tile([128, 8 * 128], F32R)
            wk_sb = cpool.tile([128, 8 * 128], F32R)
            wv_sb = cpool.tile([128, 8 * 128], F32R)
            for w_sb, w in ((wq_sb, wq), (wk_sb, wk), (wv_sb, wv)):
                # w_sb[:, dk*128+j] = w[dk*128+p, j]
                nc.sync.dma_start(
                    w_sb[:, :].rearrange("p (dk j) -> p dk j", dk=8),
                    w[:, :].rearrange("(dk p) j -> p dk j", dk=8),
                )
            wo0_sb = cpool.tile([64, D], F32R)
            wo1_sb = cpool.tile([64, D], F32R)
            nc.sync.dma_start(wo0_sb[:, :], wo0[:, :])
            nc.sync.dma_start(wo1_sb[:, :], wo1[:, :])
            bq_sb = cpool.tile([128, 1], F32)
            bk_sb = cpool.tile([128, 1], F32)
            bv_sb = cpool.tile([128, 1], F32)
            nc.sync.dma_start(bq_sb[:, :], bq[:, :])
            nc.sync.dma_start(bk_sb[:, :], bk[:, :])
            nc.sync.dma_start(bv_sb[:, :], bv[:, :])
            masks_sb = cpool.tile([128, 4 * CH], F32R)
            nc.sync.dma_start(masks_sb[:, :], masks[:, :])
            ident_sb = cpool.tile([128, 128], F32R)
            nc.sync.dma_start(ident_sb[:, :], ident[:, :])
            ones_sb = cpool.tile([128, 128], F32R)
            nc.sync.dma_start(ones_sb[:, :], ones[:, :])

            def emit_body():
                _emit(nc, tc, sb, ps, locals_)

            locals_ = dict(
                wq_sb=wq_sb, wk_sb=wk_sb, wv_sb=wv_sb, wo0_sb=wo0_sb,
                wo1_sb=wo1_sb, bq_sb=bq_sb, bk_sb=bk_sb, bv_sb=bv_sb,
                masks_sb=masks_sb, ident_sb=ident_sb, ones_sb=ones_sb,
                xT=xT, out=out)

            if reps > 1:
                with tc.For_i(0, reps, 1):
                    emit_body()
            else:
                emit_body()
    nc.compile()
    return nc


def _emit(nc, tc, sb, ps, env):
    wq_sb = env["wq_sb"]; wk_sb = env["wk_sb"]; wv_sb = env["wv_sb"]
    wo0_sb = env["wo0_sb"]; wo1_sb = env["wo1_sb"]
    bq_sb = env["bq_sb"]; bk_sb = env["bk_sb"]; bv_sb = env["bv_sb"]
    masks_sb = env["masks_sb"]; ident_sb = env["ident_sb"]
    ones_sb = env["ones_sb"]; xT = env["xT"]; out = env["out"]
    if True:
            for b in range(B):
                r0 = b * S

                # ---- QKV projections: qT/kT/vT [128(2 heads x 64), S] ----
                qT = sb.tile([128, S], F32R, tag="qT", bufs=2, name=f"qT{b}")
                kT = sb.tile([128, S], F32R, tag="kT", bufs=2, name=f"kT{b}")
                vT = sb.tile([128, S], F32R, tag="vT", bufs=1, name=f"vT{b}")
                for c in range(NC_CH):
                    xt = sb.tile([128, 8 * CH], F32R, tag="xt", bufs=3,
                                 name=f"xt{b}_{c}")
                    nc.sync.dma_start(
                        xt[:, :].rearrange("p (dk j) -> p dk j", dk=8),
                        xT[:, r0 + c * CH: r0 + (c + 1) * CH]
                        .rearrange("(dk p) j -> p dk j", dk=8),
                    )
                    for w_sb, dest, bias in ((wq_sb, qT, bq_sb),
                                             (wk_sb, kT, bk_sb),
                                             (wv_sb, vT, bv_sb)):
                        pp = ps.tile([128, 2 * CH], F32, tag="st", name=f"pp{b}{c}")
                        for dk in range(8):
                            nc.tensor.matmul(
                                pp[:, 0:CH],
                                w_sb[:, dk * 128:(dk + 1) * 128],
                                xt[:, dk * CH:(dk + 1) * CH],
                                start=(dk == 0), stop=(dk == 7),
                            )
                        nc.vector.tensor_scalar_add(
                            dest[:, c * CH:(c + 1) * CH], pp[:, 0:CH], bias[:, :])

                # ---- v1 per head: [k-tile rows 128, 16*(64 v | 1 one)] ----
                v1s = []
                for h in range(2):
                    v1 = sb.tile([128, NKT * 65], F32R, tag="v1", bufs=2,
                                 name=f"v1_{b}_{h}")
                    ones_dst = bass.AP(
                        v1.tensor, v1.offset + 64,
                        [v1.ap[0]] + [[65, NKT]])
                    nc.vector.tensor_copy(ones_dst, ones_sb[:, 0:NKT])
                    for rt in range(NKT):
                        pv = ps.tile([128, 64], F32R, tag="pv", name=f"pv{b}{h}{rt}")
                        nc.tensor.transpose(
                            pv[:, :],
                            vT[h * 64:(h + 1) * 64, rt * 128:(rt + 1) * 128],
                            ident_sb[h * 64:(h + 1) * 64, h * 64:(h + 1) * 64],
                        )
                        nc.vector.tensor_copy(
                            v1[:, rt * 65: rt * 65 + 64], pv[:, :])
                    v1s.append(v1)

                # ---- attention + normalize, per head / q-chunk ----
                oT = sb.tile([64, 2 * S], F32R, tag="oT", bufs=2, name=f"oT{b}")
                for h in range(2):
                    hp = h * 64
                    v1 = v1s[h]
                    for c in range(NC_CH):
                        qs = c * CH
                        nk = 4 * (c + 1)
                        po = ps.tile([128, CH], F32, tag="po", name=f"po{b}{h}{c}")
                        for kp in range(nk // 2):
                            st = ps.tile([128, 2 * CH], F32, tag="st",
                                         name=f"st{b}{h}{c}{kp}")
                            for j in range(2):
                                kt = 2 * kp + j
                                nc.tensor.matmul(
                                    st[:, j * CH:(j + 1) * CH],
                                    kT[hp:hp + 64, kt * 128:(kt + 1) * 128],
                                    qT[hp:hp + 64, qs:qs + CH],
                                    start=True, stop=True,
                                )
                            pt = sb.tile([128, 2 * CH], F32R, tag="pt", bufs=4,
                                         name=f"pt{b}{h}{c}{kp}")
                            nc.scalar.activation(pt[:, :], st[:, :], EXP,
                                                 scale=SCALE)
                            for j in range(2):
                                kt = 2 * kp + j
                                m = kt - 4 * c
                                if m >= 0:
                                    nc.vector.tensor_tensor(
                                        pt[:, j * CH:(j + 1) * CH],
                                        pt[:, j * CH:(j + 1) * CH],
                                        masks_sb[:, m * CH:(m + 1) * CH],
                                        MUL,
                                    )
                            for j in range(2):
                                kt = 2 * kp + j
                                nc.tensor.matmul(
                                    po[0:65, :],
                                    v1[:, kt * 65:(kt + 1) * 65],
                                    pt[:, j * CH:(j + 1) * CH],
                                    start=(kt == 0), stop=(kt == nk - 1),
                                )
                        # normalize: oT[0:64, h*S+qs:+CH] = po[0:64] / po[64]
                        rec = sb.tile([128, CH], F32, tag="rec", bufs=2,
                                      name=f"rec{b}{h}{c}")
                        nc.vector.reciprocal(rec[64:65, :], po[64:65, :])
                        recr = sb.tile([128, CH], F32R, tag="recr", bufs=2,
                                       name=f"recr{b}{h}{c}")
                        nc.vector.tensor_copy(recr[64:65, :], rec[64:65, :])
                        bc = ps.tile([128, CH], F32, tag="bc", name=f"bc{b}{h}{c}")
                        nc.tensor.matmul(
                            bc[:, :],
                            ones_sb[64:65, 0:128],
                            recr[64:65, :],
                            start=True, stop=True,
                        )
                        bcs = sb.tile([64, CH], F32, tag="bcs", bufs=2,
                                      name=f"bcs{b}{h}{c}")
                        nc.scalar.copy(bcs[:, :], bc[0:64, :])
                        nc.vector.tensor_tensor(
                            oT[0:64, h * S + qs: h * S + qs + CH],
                            po[0:64, :], bcs[:, :], MUL)

                # ---- output projection: out[r0+qt*128 :, :] partial ----
                for qt in range(NQT):
                    for ch in range(2):
                        pf = ps.tile([128, CH], F32, tag="po", name=f"pf{b}{qt}{ch}")
                        nc.tensor.matmul(
                            pf[:, :],
                            oT[0:64, qt * 128:(qt + 1) * 128],
                            wo0_sb[:, ch * CH:(ch + 1) * CH],
                            start=True, stop=False,
                        )
                        nc.tensor.matmul(
                            pf[:, :],
                            oT[0:64, S + qt * 128: S + (qt + 1) * 128],
                            wo1_sb[:, ch * CH:(ch + 1) * CH],
                            start=False, stop=True,
                        )
                        os_ = sb.tile([128, CH], F32, tag="os", bufs=3,
                                      name=f"os{b}{qt}{ch}")
                        nc.vector.tensor_copy(os_[:, :], pf[:, :])
                        nc.sync.dma_start(
                            out[r0 + qt * 128: r0 + (qt + 1) * 128,
                                ch * CH:(ch + 1) * CH],
                            os_[:, :],
                        )


_NC_CACHE = None


def _get_nc():
    global _NC_CACHE
    if _NC_CACHE is None:
        _NC_CACHE = _build_nc()
    return _NC_CACHE


def kernel(x, Wq, bq, Wk, bk, Wv, bv, Wo, bo):
    global LAST_RESULTS, LAST_IN_MAPS
    x = np.asarray(x, dtype=np.float32)
    Wq = np.asarray(Wq, dtype=np.float32)
    Wk = np.asarray(Wk, dtype=np.float32)
    Wv = np.asarray(Wv, dtype=np.float32)
    Wo = np.asarray(Wo, dtype=np.float32)
    bq = np.asarray(bq, dtype=np.float32)
    bk = np.asarray(bk, dtype=np.float32)
    bv = np.asarray(bv, dtype=np.float32)
    bo = np.asarray(bo, dtype=np.float32)

    xTr = round_fp32r(x.reshape(ROWS, D).T)

    # masks[m][p, f] = 1 if f >= 128*m + p else 0   (m = kt - 4c)
    p = np.arange(128)[:, None]
    f = np.arange(CH)[None, :]
    masks = np.concatenate(
        [(f >= 128 * m + p).astype(np.float32) for m in range(4)], axis=1)
    ident = np.eye(128, dtype=np.float32)
    ones = np.ones((128, 128), dtype=np.float32)

    in_maps = []
    for core in range(NB):
        h0, h1 = 2 * core, 2 * core + 1
        in_maps.append(dict(
            xT=xTr,
            wq=round_fp32r(np.concatenate([Wq[h0], Wq[h1]], axis=1)),
            wk=round_fp32r(np.concatenate([Wk[h0], Wk[h1]], axis=1)),
            wv=round_fp32r(np.concatenate([Wv[h0], Wv[h1]], axis=1)),
            wo0=round_fp32r(Wo[128 * core: 128 * core + 64]),
            wo1=round_fp32r(Wo[128 * core + 64: 128 * core + 128]),
            bq=np.concatenate([bq[h0], bq[h1]])[:, None].astype(np.float32),
            bk=np.concatenate([bk[h0], bk[h1]])[:, None].astype(np.float32),
            bv=np.concatenate([bv[h0], bv[h1]])[:, None].astype(np.float32),
            masks=masks, ident=ident, ones=ones,
        ))

    LAST_IN_MAPS = in_maps
    nc = _get_nc()
    kwargs = {}
    if TRACE:
        kwargs = dict(trace=True, trace_cores=list(range(NB)))
    res = run_bass_kernel_spmd(nc, in_maps, core_ids=list(range(NB)), **kwargs)
    LAST_RESULTS = res

    acc = res.results[0]["out"].astype(np.float32).copy()
    for core in range(1, NB):
        acc += res.results[core]["out"]
    acc += bo[None, :]
    return acc.reshape(B, S, D)


# revision 10
# speedup vs baseline: 1.5485x; 1.5485x over previous
"""Multi-head attention (B=4, S=2048, D=1024, H=16, HS=64, causal) on 8 trn2 cores.

Strategy: tensor-parallel over heads (2 heads per core), x replicated.
Per core: QKV projections (fp32r matmuls), causal attention with
transposed-scores softmax (no max-subtraction; scores are O(1) by
construction), output projection of the local head pair -> partial
[B*S, D]. Host sums the 8 partials (the Wo contraction over heads).

All matmuls run in float32r (fp32 with 12-bit-significand rounding,
bit-compatible with fp32). Moving free dim is kept at 256: measured on
HW, fp32r streams 1 cycle/row at N<=256 (~148 ns/matmul) but ~2
cycles/row at N=512.
"""

import sys

sys.path.insert(0, "/opt/trn_rl_repo")

import numpy as np

import concourse.bacc as bacc
import concourse.bass as bass
import concourse.mybir as mybir
import concourse.tile as tile
from concourse.bass_utils import run_bass_kernel_spmd

F32 = mybir.dt.float32
F32R = mybir.dt.float32r
EXP = mybir.ActivationFunctionType.Exp
MUL = mybir.AluOpType.mult

B, S, D, H, HS = 4, 2048, 1024, 16, 64
ROWS = B * S                      # 8192
NB = 8                            # cores
SCALE = 1.0 / float(np.sqrt(HS))  # 0.125
CH = 256                          # matmul moving width (fp32r fast path)
NC_CH = S // CH                   # 8 q/row chunks per batch
NKT = S // 128                    # 16 k-tiles per batch
NQT = S // 128                    # 16 row-tiles per batch

# knobs for test.py
TRACE = False
LAST_RESULTS = None
LAST_IN_MAPS = None


def round_fp32r(a: np.ndarray) -> np.ndarray:
    """Round fp32 to fp32r (12-bit significand, round-to-nearest-even)."""
    u = np.ascontiguousarray(a, dtype=np.float32).view(np.uint32)
    low = u & np.uint32(0xFFF)
    hi = u >> np.uint32(12)
    rnd = (low > 0x800) | ((low == 0x800) & ((hi & 1) == 1))
    return ((hi + rnd.astype(np.uint32)) << np.uint32(12)).view(np.float32)


def _build_nc(reps: int = 1):
    nc = bacc.Bacc()

    xT = nc.declare_dram_parameter("xT", [D, ROWS], F32R, isOutput=False)
    wq = nc.declare_dram_parameter("wq", [D, 128], F32R, isOutput=False)
    wk = nc.declare_dram_parameter("wk", [D, 128], F32R, isOutput=False)
    wv = nc.declare_dram_parameter("wv", [D, 128], F32R, isOutput=False)
    wo0 = nc.declare_dram_parameter("wo0", [64, D], F32R, isOutput=False)
    wo1 = nc.declare_dram_parameter("wo1", [64, D], F32R, isOutput=False)
    bq = nc.declare_dram_parameter("bq", [128, 1], F32, isOutput=False)
    bk = nc.declare_dram_parameter("bk", [128, 1], F32, isOutput=False)
    bv = nc.declare_dram_parameter("bv", [128, 1], F32, isOutput=False)
    masks = nc.declare_dram_parameter("masks", [128, 2 * CH], F32R, isOutput=False)
    ident = nc.declare_dram_parameter("ident", [128, 128], F32R, isOutput=False)
    ones = nc.declare_dram_parameter("ones", [128, 128], F32R, isOutput=False)
    out = nc.declare_dram_parameter("out", [ROWS, D], F32, isOutput=True)

    with tile.TileContext(nc) as tc:
        with tc.tile_pool(name="const", bufs=1) as cpool, \
             tc.tile_pool(name="sb", bufs=1) as sb, \
             tc.tile_pool(name="ps", bufs=1, space="PSUM") as ps:

            # ---- persistent constants ----
            wq_sb = cpool.tile([128, 8 * 128], F32R)
            wk_sb = cpool.tile([128, 8 * 128], F32R)
            wv_sb = cpool.tile([128, 8 * 128], F32R)
            for w_sb, w in ((wq_sb, wq), (wk_sb, wk), (wv_sb, wv)):
                nc.sync.dma_start(
                    w_sb[:, :].rearrange("p (dk j) -> p dk j", dk=8),
                    w[:, :].rearrange("(dk p) j -> p dk j", dk=8),
                )
            wo0_sb = cpool.tile([64, D], F32R)
            wo1_sb = cpool.tile([64, D], F32R)
            nc.sync.dma_start(wo0_sb[:, :], wo0[:, :])
            nc.sync.dma_start(wo1_sb[:, :], wo1[:, :])
            bq_sb = cpool.tile([128, 1], F32)
            bk_sb = cpool.tile([128, 1], F32)
            bv_sb = cpool.tile([128, 1], F32)
            nc.sync.dma_start(bq_sb[:, :], bq[:, :])
            nc.sync.dma_start(bk_sb[:, :], bk[:, :])
            nc.sync.dma_start(bv_sb[:, :], bv[:, :])
            masks_sb = cpool.tile([128, 2 * CH], F32R)
            nc.sync.dma_start(masks_sb[:, :], masks[:, :])
            ident_sb = cpool.tile([128, 128], F32R)
            nc.sync.dma_start(ident_sb[:, :], ident[:, :])
            ones_sb = cpool.tile([128, 128], F32R)
            nc.sync.dma_start(ones_sb[:, :], ones[:, :])

            env = dict(
                wq_sb=wq_sb, wk_sb=wk_sb, wv_sb=wv_sb, wo0_sb=wo0_sb,
                wo1_sb=wo1_sb, bq_sb=bq_sb, bk_sb=bk_sb, bv_sb=bv_sb,
                masks_sb=masks_sb, ident_sb=ident_sb, ones_sb=ones_sb,
                xT=xT, out=out)

            if reps > 1:
                with tc.For_i(0, reps, 1):
                    _emit(nc, tc, sb, ps, env)
            else:
                _emit(nc, tc, sb, ps, env)
    nc.compile()
    return nc


def _emit(nc, tc, sb, ps, env):
    wq_sb = env["wq_sb"]; wk_sb = env["wk_sb"]; wv_sb = env["wv_sb"]
    wo0_sb = env["wo0_sb"]; wo1_sb = env["wo1_sb"]
    bq_sb = env["bq_sb"]; bk_sb = env["bk_sb"]; bv_sb = env["bv_sb"]
    masks_sb = env["masks_sb"]; ident_sb = env["ident_sb"]
    ones_sb = env["ones_sb"]; xT = env["xT"]; out = env["out"]

    for b in range(B):
        r0 = b * S

        # ---- QKV projections: qT/kT/vT [128(2 heads x 64), S] ----
        qT = sb.tile([128, S], F32R, tag="qT", bufs=2, name=f"qT{b}")
        kT = sb.tile([128, S], F32R, tag="kT", bufs=2, name=f"kT{b}")
        vT = sb.tile([128, S], F32R, tag="vT", bufs=1, name=f"vT{b}")
        # process chunk pairs (2 x 256 = one [128,512] psum + one copy)
        for cp in range(NC_CH // 2):
            xt = sb.tile([128, 8 * 2 * CH], F32R, tag="xt", bufs=3,
                         name=f"xt{b}_{cp}")
            nc.sync.dma_start(
                xt[:, :].rearrange("p (dk j) -> p dk j", dk=8),
                xT[:, r0 + cp * 2 * CH: r0 + (cp + 1) * 2 * CH]
                .rearrange("(dk p) j -> p dk j", dk=8),
            )
            for w_sb, dest, bias in ((wq_sb, qT, bq_sb),
                                     (wk_sb, kT, bk_sb),
                                     (wv_sb, vT, bv_sb)):
                pp = ps.tile([128, 512], F32, tag="st", bufs=2, name=f"pp{b}{cp}")
                for half in range(2):
                    for dk in range(8):
                        nc.tensor.matmul(
                            pp[:, half * CH:(half + 1) * CH],
                            w_sb[:, dk * 128:(dk + 1) * 128],
                            xt[:, dk * 2 * CH + half * CH:
                               dk * 2 * CH + (half + 1) * CH],
                            start=(dk == 0), stop=(dk == 7),
                        )
                nc.vector.tensor_scalar_add(
                    dest[:, cp * 2 * CH:(cp + 1) * 2 * CH], pp[:, :],
                    bias[:, :])

        # ---- v1 per head: [k-tile rows 128, 16*(64 v | 1 one)] ----
        v1s = []
        for h in range(2):
            v1 = sb.tile([128, NKT * 65], F32R, tag="v1", bufs=2,
                         name=f"v1_{b}_{h}")
            ones_dst = bass.AP(
                v1.tensor, v1.offset + 64,
                [v1.ap[0]] + [[65, NKT]])
            nc.vector.tensor_copy(ones_dst, ones_sb[:, 0:NKT])
            for rt in range(NKT):
                pv = ps.tile([128, 64], F32R, tag="bc", bufs=2, name=f"pv{b}{h}{rt}")
                nc.tensor.transpose(
                    pv[:, :],
                    vT[h * 64:(h + 1) * 64, rt * 128:(rt + 1) * 128],
                    ident_sb[h * 64:(h + 1) * 64, h * 64:(h + 1) * 64],
                )
                nc.vector.tensor_copy(
                    v1[:, rt * 65: rt * 65 + 64], pv[:, :])
            v1s.append(v1)

        # ---- attention + normalize, per head / q-chunk (256 wide) ----
        oT = sb.tile([64, 2 * S], F32R, tag="oT", bufs=2, name=f"oT{b}")
        for h in range(2):
            hp = h * 64
            v1 = v1s[h]
            for c in range(NC_CH):
                qs = c * CH
                nk = 2 * (c + 1)          # k-tiles 0..nk-1 (last 2 diagonal)
                po = ps.tile([128, CH], F32, tag="po", bufs=2, name=f"po{b}{h}{c}")
                for g in range((nk + 3) // 4):   # groups of 4 k-tiles
                    k0 = 4 * g
                    gn = min(4, nk - k0)
                    st = ps.tile([128, 1024], F32, tag="st", bufs=2,
                                 name=f"st{b}{h}{c}{g}")
                    for j in range(gn):
                        nc.tensor.matmul(
                            st[:, j * CH:(j + 1) * CH],
                            kT[hp:hp + 64, (k0 + j) * 128:(k0 + j + 1) * 128],
                            qT[hp:hp + 64, qs:qs + CH],
                            start=True, stop=True,
                        )
                    pt = sb.tile([128, 1024], F32R, tag="pt", bufs=4,
                                 name=f"pt{b}{h}{c}{g}")
                    nc.scalar.activation(pt[:, 0:gn * CH], st[:, 0:gn * CH],
                                         EXP, scale=SCALE)
                    for j in range(gn):
                        m = (k0 + j) - 2 * c
                        if m >= 0:
                            nc.vector.tensor_tensor(
                                pt[:, j * CH:(j + 1) * CH],
                                pt[:, j * CH:(j + 1) * CH],
                                masks_sb[:, m * CH:(m + 1) * CH],
                                MUL,
                            )
                    for j in range(gn):
                        kt = k0 + j
                        nc.tensor.matmul(
                            po[0:65, :],
                            v1[:, kt * 65:(kt + 1) * 65],
                            pt[:, j * CH:(j + 1) * CH],
                            start=(kt == 0), stop=(kt == nk - 1),
                        )
                # normalize: oT[0:64, h*S+qs:+CH] = po[0:64] / po[64]
                rec = sb.tile([128, CH], F32, tag="rec", bufs=2,
                              name=f"rec{b}{h}{c}")
                nc.vector.reciprocal(rec[64:65, :], po[64:65, :])
                recr = sb.tile([128, CH], F32R, tag="recr", bufs=2,
                               name=f"recr{b}{h}{c}")
                nc.vector.tensor_copy(recr[64:65, :], rec[64:65, :])
                bc = ps.tile([128, CH], F32, tag="bc", bufs=2, name=f"bc{b}{h}{c}")
                nc.tensor.matmul(
                    bc[:, :],
                    ones_sb[64:65, 0:128],
                    recr[64:65, :],
                    start=True, stop=True,
                )
                bcs = sb.tile([64, CH], F32, tag="bcs", bufs=2,
                              name=f"bcs{b}{h}{c}")
                nc.scalar.copy(bcs[:, :], bc[0:64, :])
                nc.vector.tensor_tensor(
                    oT[0:64, h * S + qs: h * S + qs + CH],
                    po[0:64, :], bcs[:, :], MUL)

        # ---- output projection: out[r0+qt*128 :, :] partial ----
        for qt in range(NQT):
            for chp in range(2):          # D-chunk pairs (2 x 256)
                pf = ps.tile([128, 512], F32, tag="po", bufs=2, name=f"pf{b}{qt}{chp}")
                for half in range(2):
                    d0 = chp * 512 + half * CH
                    nc.tensor.matmul(
                        pf[:, half * CH:(half + 1) * CH],
                        oT[0:64, qt * 128:(qt + 1) * 128],
                        wo0_sb[:, d0:d0 + CH],
                        start=True, stop=False,
                    )
                    nc.tensor.matmul(
                        pf[:, half * CH:(half + 1) * CH],
                        oT[0:64, S + qt * 128: S + (qt + 1) * 128],
                        wo1_sb[:, d0:d0 + CH],
                        start=False, stop=True,
                    )
                os_ = sb.tile([128, 512], F32, tag="os", bufs=3,
                              name=f"os{b}{qt}{chp}")
                nc.vector.tensor_copy(os_[:, :], pf[:, :])
                nc.sync.dma_start(
                    out[r0 + qt * 128: r0 + (qt + 1) * 128,
                        chp * 512:(chp + 1) * 512],
                    os_[:, :],
                )


_NC_CACHE = None


def _get_nc():
    global _NC_CACHE
    if _NC_CACHE is None:
        _NC_CACHE = _build_nc()
    return _NC_CACHE


def kernel(x, Wq, bq, Wk, bk, Wv, bv, Wo, bo):
    global LAST_RESULTS, LAST_IN_MAPS
    x = np.asarray(x, dtype=np.float32)
    Wq = np.asarray(Wq, dtype=np.float32)
    Wk = np.asarray(Wk, dtype=np.float32)
    Wv = np.asarray(Wv, dtype=np.float32)
    Wo = np.asarray(Wo, dtype=np.float32)
    bq = np.asarray(bq, dtype=np.float32)
    bk = np.asarray(bk, dtype=np.float32)
    bv = np.asarray(bv, dtype=np.float32)
    bo = np.asarray(bo, dtype=np.float32)

    xTr = round_fp32r(x.reshape(ROWS, D).T)

    # masks[m][p, f] = 1 if f >= 128*m + p else 0   (m = kt - 2c)
    p = np.arange(128)[:, None]
    f = np.arange(CH)[None, :]
    masks = np.concatenate(
        [(f >= 128 * m + p).astype(np.float32) for m in range(2)], axis=1)
    ident = np.eye(128, dtype=np.float32)
    ones = np.ones((128, 128), dtype=np.float32)

    in_maps = []
    for core in range(NB):
        h0, h1 = 2 * core, 2 * core + 1
        in_maps.append(dict(
            xT=xTr,
            wq=round_fp32r(np.concatenate([Wq[h0], Wq[h1]], axis=1)),
            wk=round_fp32r(np.concatenate([Wk[h0], Wk[h1]], axis=1)),
            wv=round_fp32r(np.concatenate([Wv[h0], Wv[h1]], axis=1)),
            wo0=round_fp32r(Wo[128 * core: 128 * core + 64]),
            wo1=round_fp32r(Wo[128 * core + 64: 128 * core + 128]),
            bq=np.concatenate([bq[h0], bq[h1]])[:, None].astype(np.float32),
            bk=np.concatenate([bk[h0], bk[h1]])[:, None].astype(np.float32),
            bv=np.concatenate([bv[h0], bv[h1]])[:, None].astype(np.float32),
            masks=masks, ident=ident, ones=ones,
        ))

    LAST_IN_MAPS = in_maps
    nc = _get_nc()
    kwargs = {}
    if TRACE:
        kwargs = dict(trace=True, trace_cores=list(range(NB)))
    res = run_bass_kernel_spmd(nc, in_maps, core_ids=list(range(NB)), **kwargs)
    LAST_RESULTS = res

    acc = res.results[0]["out"].astype(np.float32).copy()
    for core in range(1, NB):
        acc += res.results[core]["out"]
    acc += bo[None, :]
    return acc.reshape(B, S, D)
